# revision 7
# baseline (speedup 1.0000x reference)
"""Single-head attention (B=4, S=4096, D=1024, K=128) on 8 TRN2 NeuronCores.

Sharding: batch (4) x query-half (2) = 8 shards. Each core computes K/V
projections over the full sequence of its batch element and attention for
its 2048 query rows. No collectives needed.

Per-core layout (everything transposed so no on-chip attn transposes):
  xt   [1024, 4096]  X^T for batch b, with the core's q-half columns first
  KT/VT[128, 4096]   k-dim on partitions
  QT   [128, 2048]
  V    [s, kd] via PE transpose of VT
  ST   [s, q] score tiles = KT_tile.T @ QT   (softmax keys on partitions+tiles)
  OT   [kd, q] = sum_s V_tile.T @ exp(ST)    (host transposes back)
No-max-subtraction softmax: |scores/sqrt(128)| <= ~19 for this data, exp and
row sums stay well inside fp32 range.
"""
import sys
import types
import numpy as np

B, S, D, KD = 4, 4096, 1024, 128
QH = S // 2              # queries per core
SCALE = 1.0 / np.sqrt(KD)
N_SLAB = 8               # seq slabs of 512 for projections
SLAB = S // N_SLAB       # 512
N_ST = S // 128          # 32 s-tiles of 128
QT_TILE = 512            # q tile width
N_QT = QH // QT_TILE     # 4
SGRP = 3                 # s-tiles per exp group (3 PSUM banks, x2 buffered)

_MAX_WAITS = 1


def _install_shims():
    """Environment fixes: NTFF profiling hook under axon + walrus sync-wait cap."""
    import concourse.bass_utils as bu
    try:
        import antenv.axon_hooks  # noqa: F401
    except ImportError:
        try:
            import trn_agent_boot.trn_boot as tb
            hook = tb._ntff_profile_via_ctypes('/opt/axon/libaxon_pjrt.so')
        except Exception:
            hook = None
        mod = types.ModuleType('antenv.axon_hooks')
        mod.get_axon_ntff_profile_hook = lambda: hook
        mod.set_axon_ntff_profile_hook = lambda h: None
        sys.modules['antenv.axon_hooks'] = mod
        import antenv
        antenv.axon_hooks = mod
    bu.upload_artifacts = lambda tmpdir: tmpdir

    import concourse.tile as tile
    import concourse.mybir as mybir
    from concourse.vector_clock import ScopedClock

    def _drain_and_barrier(self, tick_clock, wait_clock):
        nc = self.nc
        # The walrus build here only accepts 1 sync-wait per CTRL instruction;
        # spread the tail drain's waits over preceding single-wait NOPs.
        nops = [nc.sync.nop(nofuse=True, hint=f"predrain{i}") for i in range(30)]
        drain_inst = nc.sync.drain()
        wait_clock.add_sem_waits(
            drain_inst.ins, ScopedClock({None: tick_clock.global_clock})
        )
        waits = list(drain_inst.ins.sync_info.on_wait or [])
        if len(waits) > _MAX_WAITS:
            drain_inst.ins.sync_info.on_wait = waits[:_MAX_WAITS - 1] if _MAX_WAITS > 1 else []
            rest = waits[_MAX_WAITS - 1:] if _MAX_WAITS > 1 else waits
            for i, nop in enumerate(nops):
                chunk = rest[i * _MAX_WAITS:(i + 1) * _MAX_WAITS]
                if chunk:
                    if nop.ins.sync_info is None:
                        nop.ins.sync_info = mybir.SyncInfo(on_wait=chunk, on_update=[])
                    else:
                        nop.ins.sync_info.on_wait = chunk
        nc.all_engine_barrier()
        assert self.sems is not None
        popped = nc._tile_sem_poison_stack.pop()
        assert popped is self._sem_poison
        nc.clear_and_free_semaphores(list(self.sems.allocated().values()))
        nc.all_engine_barrier()

    tile.TileContext._drain_and_barrier = _drain_and_barrier


def _split_waits(nc):
    """This walrus build accepts at most 1 sync-wait per instruction; hoist
    excess waits onto same-engine NoOps inserted immediately before."""
    import concourse.mybir as mybir
    ctr = [0]
    for fn in nc.m.functions:
        for blk in fn.blocks:
            insts = blk.instructions
            out = []
            for inst in insts:
                si = getattr(inst, "sync_info", None)
                waits = list(si.on_wait) if si is not None and si.on_wait else []
                if len(waits) > 1:
                    for w in waits[1:]:
                        ctr[0] += 1
                        nop = mybir.InstNoOp(name=f"I-ws{ctr[0]}", ins=[], outs=[])
                        nop.engine = inst.engine
                        nop.sync_info = mybir.SyncInfo(on_wait=[w], on_update=[])
                        out.append(nop)
                    si.on_wait = waits[:1]
                out.append(inst)
            if len(out) != len(insts):
                insts.clear()
                insts.extend(out)


def build_graph():
    import concourse.bass as bass
    import concourse.mybir as mybir
    import concourse.tile as tile
    dt = mybir.dt
    f32, f32r = dt.float32, dt.float32r
    EXP = mybir.ActivationFunctionType.Exp

    nc = bass.Bass()
    xt = nc.declare_dram_parameter("xt", [D, S], f32r, isOutput=False).ap()
    wq = nc.declare_dram_parameter("wq", [D, KD], f32r, isOutput=False).ap()
    wk = nc.declare_dram_parameter("wk", [D, KD], f32r, isOutput=False).ap()
    wv = nc.declare_dram_parameter("wv", [D, KD], f32r, isOutput=False).ap()
    ident = nc.declare_dram_parameter("ident", [128, 128], f32, isOutput=False).ap()
    ones_h = nc.declare_dram_parameter("ones_h", [128, 1], f32r, isOutput=False).ap()
    out = nc.declare_dram_parameter("out", [KD, QH], f32, isOutput=True).ap()

    ND = D // 128  # 8 d-tiles

    with tile.TileContext(nc) as tc:
        with (
            tc.tile_pool(name="w", bufs=4) as wp,
            tc.tile_pool(name="kt", bufs=1) as ktp,
            tc.tile_pool(name="qt", bufs=1) as qtp,
            tc.tile_pool(name="v", bufs=1) as vp,
            tc.tile_pool(name="ones", bufs=1) as onesp,
        ):
            # ---- resident tensors ----
            w_sb = {}
            for name, w in (("wq", wq), ("wk", wk), ("wv", wv)):
                t = wp.tile([128, D], f32r, tag="w")
                nc.sync.dma_start(
                    t[:].rearrange("p (t k) -> p t k", t=ND),
                    w.rearrange("(t p) k -> p t k", p=128),
                )
                w_sb[name] = t
            id_sb = wp.tile([128, 128], f32, tag="ident")
            nc.sync.dma_start(id_sb[:], ident)
            kt_sb = ktp.tile([128, S], f32r)
            qt_sb = qtp.tile([128, QH], f32r)
            v_sb = vp.tile([128, S], f32r)   # v_sb[:, st*128: ] = V[s-tile] as [s, kd]
            ones_sb = onesp.tile([128, 1], f32r)
            nc.sync.dma_start(ones_sb[:], ones_h)

            # ---- phase P: projections, streamed over seq slabs ----
            with (
                tc.tile_pool(name="xts", bufs=2) as xtp,
                tc.tile_pool(name="pp", bufs=6, space="PSUM") as pp,
                tc.tile_pool(name="vtp", bufs=2, space="PSUM") as vtp,
            ):
                for j in range(N_SLAB):
                    xts = xtp.tile([128, D // 128 * SLAB], f32r, tag="xts")
                    nc.sync.dma_start(
                        xts[:].rearrange("p (t s) -> p t s", t=ND),
                        xt[:, j * SLAB:(j + 1) * SLAB].rearrange(
                            "(t p) s -> p t s", p=128),
                    )
                    projs = [("wk", kt_sb), ("wv", None), ("wq", qt_sb)]
                    if j >= N_SLAB // 2:
                        projs = projs[:2]  # q-half columns are slabs 0..3 only
                    for name, dst in projs:
                        ps = pp.tile([128, SLAB], f32, tag="pp")
                        for d in range(ND):
                            nc.tensor.matmul(
                                ps[:],
                                w_sb[name][:, d * 128:(d + 1) * 128],
                                xts[:, d * SLAB:(d + 1) * SLAB],
                                start=(d == 0), stop=(d == ND - 1),
                            )
                        if name == "wv":
                            # transpose VT slab -> V tiles [s, kd]; needs SBUF src
                            vt_sb = xtp.tile([128, SLAB], f32, tag="vts")
                            nc.scalar.copy(vt_sb[:], ps[:])
                            for c in range(SLAB // 128):
                                st_i = j * (SLAB // 128) + c
                                tp = vtp.tile([128, 128], f32, tag="vt")
                                nc.tensor.transpose(
                                    tp[:], vt_sb[:, c * 128:(c + 1) * 128], id_sb[:])
                                nc.vector.tensor_copy(
                                    v_sb[:, st_i * 128:(st_i + 1) * 128], tp[:])
                        else:
                            nc.scalar.copy(dst[:, j * SLAB:(j + 1) * SLAB], ps[:])

            # ---- phase A: attention ----
            sgroups = []
            st0 = 0
            while st0 < N_ST:
                sgroups.append(list(range(st0, min(st0 + SGRP, N_ST))))
                st0 += SGRP

            with (
                tc.tile_pool(name="st", bufs=2, space="PSUM") as stp,
                tc.tile_pool(name="ot", bufs=2, space="PSUM") as otp,
                tc.tile_pool(name="est", bufs=3) as estp,
                tc.tile_pool(name="racc", bufs=2) as raccp,
                tc.tile_pool(name="norm", bufs=2) as normp,
                tc.tile_pool(name="osb", bufs=2) as osbp,
            ):
                for q in range(N_QT):
                    qs = slice(q * QT_TILE, (q + 1) * QT_TILE)
                    ot = otp.tile([128, QT_TILE], f32, tag="ot")
                    racc = raccp.tile([128, QT_TILE], f32r, tag="racc")
                    n_add = 0
                    for g in sgroups:
                        stps = stp.tile([128, SGRP * QT_TILE], f32, tag="st")
                        for i, st_i in enumerate(g):
                            nc.tensor.matmul(
                                stps[:, i * QT_TILE:(i + 1) * QT_TILE],
                                kt_sb[:, st_i * 128:(st_i + 1) * 128],
                                qt_sb[:, qs],
                                start=True, stop=True,
                            )
                        est = estp.tile([128, SGRP * QT_TILE], f32r, tag="est")
                        w_grp = len(g) * QT_TILE
                        nc.scalar.activation(
                            est[:, :w_grp], stps[:, :w_grp], EXP, scale=float(SCALE))
                        for i, st_i in enumerate(g):
                            sl = est[:, i * QT_TILE:(i + 1) * QT_TILE]
                            nc.tensor.matmul(
                                ot[:],
                                v_sb[:, st_i * 128:(st_i + 1) * 128],
                                sl,
                                start=(st_i == 0), stop=(st_i == N_ST - 1),
                            )
                            if n_add == 0:
                                first_sl = sl
                            elif n_add == 1:
                                nc.vector.tensor_add(racc[:], first_sl, sl)
                            else:
                                nc.vector.tensor_add(racc[:], racc[:], sl)
                            n_add += 1
                    # R[q] = sum over partitions of racc (ones-matmul), then 1/R
                    rsum = otp.tile([1, QT_TILE], f32, tag="ot")
                    nc.tensor.matmul(
                        rsum[:], ones_sb[:], racc[:],
                        start=True, stop=True)
                    rbc = normp.tile([128, QT_TILE], f32, tag="rbc")
                    nc.vector.reciprocal(rbc[0:1, :], rsum[:])
                    p = 1
                    while p < 128:  # broadcast partition 0 -> all via doubling DMAs
                        nc.sync.dma_start(rbc[p:2 * p, :], rbc[0:p, :])
                        p *= 2
                    o_sb = osbp.tile([128, QT_TILE], f32, tag="osb")
                    nc.vector.tensor_mul(o_sb[:], ot[:], rbc[:])
                    nc.sync.dma_start(out[:, qs], o_sb[:])
    _split_waits(nc)
    return nc


_CACHED = {}


def kernel(input_vec, weight_query, weight_key, weight_value):
    _install_shims()
    from concourse.bass_utils import run_bass_kernel_spmd

    x = np.ascontiguousarray(np.asarray(input_vec, dtype=np.float32))
    wq = np.ascontiguousarray(np.asarray(weight_query, dtype=np.float32))
    wk = np.ascontiguousarray(np.asarray(weight_key, dtype=np.float32))
    wv = np.ascontiguousarray(np.asarray(weight_value, dtype=np.float32))
    ident = np.eye(128, dtype=np.float32)

    if "nc" not in _CACHED:
        _CACHED["nc"] = build_graph()
    nc = _CACHED["nc"]

    in_maps = []
    for c in range(8):
        b, h = c // 2, c % 2
        XT = x[b].T  # [D, S]
        qlo, qhi = h * QH, (h + 1) * QH
        xt_c = np.concatenate([XT[:, qlo:qhi], XT[:, :qlo], XT[:, qhi:]], axis=1)
        in_maps.append({
            "xt": np.ascontiguousarray(xt_c),
            "wq": wq, "wk": wk, "wv": wv, "ident": ident,
            "ones_h": np.ones((128, 1), dtype=np.float32),
        })

    import os
    trace = bool(os.environ.get("KERNEL_TRACE"))
    res = run_bass_kernel_spmd(nc, in_maps, list(range(8)), trace=trace)
    _CACHED["last_exec_time_ns"] = res.exec_time_ns
    _CACHED["last_results"] = res

    out = np.empty((B, S, KD), dtype=np.float32)
    for c in range(8):
        b, h = c // 2, c % 2
        out[b, h * QH:(h + 1) * QH, :] = res.results[c]["out"].T
    return out
